# revision 32
# baseline (speedup 1.0000x reference)
"""Multi-head attention (B=2, S=2048, H=16, D=64) on 8 Trainium2 NeuronCores.

Head-parallel tensor parallelism: core c owns heads {2c, 2c+1} (a 128-dim
slice of the model dim): column-parallel QKV projections and local causal
attention for its 2 heads, then an AllToAll of bf16 context vectors (one
half-batch at a time, pipelined behind attention) and a full-width Wo
projection for this core's own disjoint 128-token output slices.

Shaped by trace measurements on this part:

* x loads in 8 per-token-tile DMAs and the QKV projection of tile t is
  interleaved with attention on query group t-1, so the PE starts ~20 us
  earlier than with a monolithic x load, and ACT-bound attention stretches
  overlap projection matmuls.
* Attention-times-V keeps V plus a trailing ones column as the 65-column
  stationary operand and streams the exp tile (one matmul per key block,
  N<=512): context comes out already transposed ([dims, tokens]) and the
  softmax denominator lands on PSUM partition 64.
* Scores use tile_position row pairs: each head is a K=64 matmul on its own
  row-group half of the PE array, so the two heads' score matmuls run
  concurrently (no zero-padded K=128 operands).
* exp is one ACT instruction per key block covering both heads.
* Softmax normalization happens on the *receiving* core: the AllToAll
  payload is 130 rows per peer (65 per head: 64 unnormalized ctx dims plus
  the denominator row), so the 16 denominator rows stack on the partition
  axis at the receiver where one 16-lane DVE reciprocal + a DRAM-bounced
  stride-0 broadcast + one fused multiply normalize the gathered ctx.
  (DVE reciprocal is ~8 cycles/element/lane, so sender-side row-wise
  reciprocals were 3.3 us each; gpsimd partition_broadcast and the custom
  reciprocal_approx_fast DVE op both produce wrong results on hardware.)
* Collective triggers block the GpSimd queue until the collective
  completes, so nothing else is ever placed on GpSimd, and each half-batch
  Wo projection is emitted two sections after its AllToAll was issued.
* A tiny warm-up AllToAll is issued during the load phase so the first real
  collective doesn't pay the ~23 us first-call setup on the critical path.
"""

import sys

sys.path.insert(0, "/opt/trn_rl_repo")

import ml_dtypes
import numpy as np

import concourse.bass as bass
import concourse.tile as tile
from concourse import bacc, mybir
from concourse.bass_utils import run_bass_kernel_spmd

N_CORES = 8
B, S, H, D = 2, 2048, 16, 64
E = H * D            # 1024
T = B * S            # 4096 tokens
DPC = 128            # dims (2 heads) per core
NKC = E // 128       # 8 contraction chunks for the projections
NTT = T // 512       # 8 token tiles of 512
SB = S // 128        # 16 key blocks per batch
PH = S // 2 // N_CORES  # 128 tokens per core per half-batch
CR = 130             # a2a chunk rows: 2 x (64 ctx dims + den)

F32 = mybir.dt.float32
BF16 = mybir.dt.bfloat16
AFT = mybir.ActivationFunctionType


def build_program():
    nc = bacc.Bacc("TRN2", target_bir_lowering=False, debug=False,
                   num_devices=N_CORES)

    xT = nc.dram_tensor("xT", [E, T], BF16, kind="ExternalInput").ap()
    wqT = nc.dram_tensor("wqT", [E, DPC], BF16, kind="ExternalInput").ap()
    wkT = nc.dram_tensor("wkT", [E, DPC], BF16, kind="ExternalInput").ap()
    wvT = nc.dram_tensor("wvT", [E, DPC], BF16, kind="ExternalInput").ap()
    woT = nc.dram_tensor("woT", [E, E], BF16, kind="ExternalInput").ap()
    bq = nc.dram_tensor("bq", [DPC, 1], F32, kind="ExternalInput").ap()
    bk = nc.dram_tensor("bk", [DPC, 1], F32, kind="ExternalInput").ap()
    bv = nc.dram_tensor("bv", [DPC, 1], F32, kind="ExternalInput").ap()
    bo = nc.dram_tensor("bo", [E], F32, kind="ExternalInput").ap()
    # single 128x128 lower-triangular (k_local <= q_local) mask
    tri = nc.dram_tensor("tri", [128, 128], BF16, kind="ExternalInput").ap()
    ident = nc.dram_tensor("ident", [128, 128], BF16, kind="ExternalInput").ap()
    out = nc.dram_tensor("out", [T // N_CORES, E], F32, kind="ExternalOutput").ap()

    with tile.TileContext(nc) as tc:
        with (
            tc.tile_pool(name="consts", bufs=1) as consts,
            tc.tile_pool(name="state", bufs=1) as state,
            tc.tile_pool(name="ep", bufs=4) as ep,
            tc.tile_pool(name="op", bufs=4) as op,
            tc.tile_pool(name="ps_s", bufs=2, space="PSUM") as ps_s,
            tc.tile_pool(name="ps_c", bufs=3, space="PSUM") as ps_c,
            tc.tile_pool(name="ps_t", bufs=1, space="PSUM") as ps_t,
            tc.tile_pool(name="dram", bufs=1, space="DRAM") as dram,
        ):
            # ---- warm-up collective: absorbs the first-AllToAll setup cost
            # while the DMA engines are still loading x ----------------------
            wu_s = consts.tile([128, 16], BF16)
            nc.vector.memset(wu_s[:], 0.0)
            wu_in = dram.tile([N_CORES, 16, 16], BF16, tag="wu_in", name="wu_in")
            wu_out = dram.tile([N_CORES, 16, 16], BF16, tag="wu_out",
                               name="wu_out")
            nc.sync.dma_start(out=wu_in[:], in_=wu_s[:])
            nc.gpsimd.collective_compute(
                "AllToAll",
                mybir.AluOpType.bypass,
                replica_groups=[list(range(N_CORES))],
                ins=[wu_in.opt()],
                outs=[wu_out.opt()],
            )

            # ---- constants (one DMA per tensor) ---------------------------
            def chunked(dram_ap, cols):
                # DRAM [E, cols] viewed as [p, kc, cols]: row kc*128+p
                return bass.AP(tensor=dram_ap.tensor, offset=dram_ap.offset,
                               ap=[[cols, 128], [128 * cols, NKC], [1, cols]])

            wq_sb = consts.tile([128, NKC, DPC], BF16)
            wk_sb = consts.tile([128, NKC, DPC], BF16)
            wv_sb = consts.tile([128, NKC, DPC], BF16)
            nc.sync.dma_start(out=wq_sb[:], in_=chunked(wqT, DPC))
            nc.sync.dma_start(out=wk_sb[:], in_=chunked(wkT, DPC))
            nc.sync.dma_start(out=wv_sb[:], in_=chunked(wvT, DPC))
            bq_sb = consts.tile([128, 1], F32)
            bk_sb = consts.tile([128, 1], F32)
            bv_sb = consts.tile([128, 1], F32)
            nc.sync.dma_start(out=bq_sb[:], in_=bq[:])
            nc.sync.dma_start(out=bk_sb[:], in_=bk[:])
            nc.sync.dma_start(out=bv_sb[:], in_=bv[:])
            bo_bc = consts.tile([128, E], F32)
            nc.sync.dma_start(
                out=bo_bc[:],
                in_=bass.AP(tensor=bo.tensor, offset=bo.offset,
                            ap=[[0, 128], [1, E]]),
            )
            tri_sb = consts.tile([128, 128], BF16)
            nc.sync.dma_start(out=tri_sb[:], in_=tri[:])
            id_sb = consts.tile([128, 128], BF16)
            nc.sync.dma_start(out=id_sb[:], in_=ident[:])

            # ---- x, two tiles per 512-token group.  Bulk loads go on the
            # scalar engine's HWDGE ring so they never queue ahead of the
            # small latency-critical DMAs on the sync ring (HWDGE rings
            # drain FIFO per issuing engine). -------------------------------
            x_t = []
            for tt in range(NTT):
                halves = []
                for hx in range(2):
                    xt = state.tile([128, NKC // 2, 512], BF16,
                                    name=f"x{tt}_{hx}")
                    nc.scalar.dma_start(
                        out=xt[:],
                        in_=bass.AP(
                            tensor=xT.tensor,
                            offset=xT.offset + hx * (NKC // 2) * 128 * T
                            + tt * 512,
                            ap=[[T, 128], [128 * T, NKC // 2], [1, 512]]))
                    halves.append(xt)
                x_t.append(halves)
            wo_sb = consts.tile([128, NKC, E], BF16)
            nc.scalar.dma_start(out=wo_sb[:], in_=chunked(woT, E))

            def x_chunk(tt, kc):
                return x_t[tt][kc // 4][:, kc % 4, :]

            # ---- persistent activations -----------------------------------
            qT_sb = state.tile([128, T], BF16)   # [2-head dims, tokens]
            kT_sb = state.tile([128, T], BF16)
            vT_sb = state.tile([128, T], BF16)
            # per 128-token block: [64 v-dims, ones] per head -> the AV
            # matmul's 65-column stationary operand; the ones column makes
            # PSUM row 64 the softmax denominator.
            vN_sb = state.tile([128, T // 128, 130], BF16)
            # unnormalized ctx^T + den: rows 0-63 ctx dims, row 64 den
            ctx2_sb = state.tile([65, 2, T], BF16)

            nc.vector.memset(vN_sb[:, :, 64:65], 1.0)
            nc.vector.memset(vN_sb[:, :, 129:130], 1.0)

            # ---- stage builders -------------------------------------------
            def emit_proj(tt):
                ts = slice(tt * 512, (tt + 1) * 512)
                ps_qk = ps_s.tile([128, 2, 512], F32, tag="s", name="ps_qk")
                for kc in range(NKC):
                    nc.tensor.matmul(ps_qk[:, 0, :], wq_sb[:, kc, :],
                                     x_chunk(tt, kc),
                                     start=(kc == 0), stop=(kc == NKC - 1),
                                     skip_group_check=True)
                for kc in range(NKC):
                    nc.tensor.matmul(ps_qk[:, 1, :], wk_sb[:, kc, :],
                                     x_chunk(tt, kc),
                                     start=(kc == 0), stop=(kc == NKC - 1),
                                     skip_group_check=True)
                ps_v = ps_s.tile([128, 2, 512], F32, tag="s", name="ps_v")
                for kc in range(NKC):
                    nc.tensor.matmul(ps_v[:, 0, :], wv_sb[:, kc, :],
                                     x_chunk(tt, kc),
                                     start=(kc == 0), stop=(kc == NKC - 1),
                                     skip_group_check=True)
                nc.vector.tensor_scalar_add(qT_sb[:, ts], ps_qk[:, 0, :],
                                            bq_sb[:])
                nc.vector.tensor_scalar_add(kT_sb[:, ts], ps_qk[:, 1, :],
                                            bk_sb[:])
                nc.vector.tensor_scalar_add(vT_sb[:, ts], ps_v[:, 0, :],
                                            bv_sb[:])
                for tb in range(tt * 4, tt * 4 + 4):
                    tp_ps = ps_t.tile([128, 128], BF16, tag="tp", name="tp_ps")
                    nc.tensor.transpose(
                        tp_ps[:], vT_sb[:, tb * 128:(tb + 1) * 128], id_sb[:])
                    nc.vector.tensor_copy(vN_sb[:, tb, 0:64], tp_ps[:, 0:64])
                    nc.vector.tensor_copy(vN_sb[:, tb, 65:129],
                                          tp_ps[:, 64:128])

            def emit_attn(b, qt):
                t0 = b * S
                q0 = t0 + qt * 512
                nkb = 4 * qt + 4

                def emit_scores(kb):
                    c0 = max(kb - 4 * qt, 0) * 128
                    s = ps_s.tile([128, 2, 512], F32, tag="s", name="s_ps")
                    for h in range(2):
                        d0 = h * 64
                        nc.tensor.matmul(
                            s[:, h, c0:512],
                            kT_sb[d0:d0 + 64,
                                  t0 + kb * 128:t0 + (kb + 1) * 128],
                            qT_sb[d0:d0 + 64, q0 + c0:q0 + 512],
                            start=True, stop=True, skip_group_check=True)
                    return s

                s_tiles = {0: emit_scores(0)}
                cn = [ps_c.tile([128, 512], F32, tag="cn", name=f"cn{h}")
                      for h in range(2)]
                for kb in range(nkb):
                    m = kb - 4 * qt
                    c0 = max(m, 0) * 128
                    if kb + 1 < nkb:
                        s_tiles[kb + 1] = emit_scores(kb + 1)
                    s = s_tiles.pop(kb)
                    e = ep.tile([128, 2, 512], BF16, tag="e", name="e_sb")
                    nc.scalar.activation(e[:, :, c0:512], s[:, :, c0:512],
                                         AFT.Exp, scale=0.125)
                    if m >= 0:  # triangular block on the diagonal
                        for h in range(2):
                            nc.vector.tensor_mul(e[:, h, c0:c0 + 128],
                                                 e[:, h, c0:c0 + 128],
                                                 tri_sb[:])
                    for h in range(2):
                        nc.tensor.matmul(
                            cn[h][0:65, c0:512],
                            vN_sb[:, b * SB + kb, 65 * h:65 * h + 65],
                            e[:, h, c0:512],
                            start=(kb == 0), stop=(kb == nkb - 1),
                            skip_group_check=True)

                # stage unnormalized ctx + den rows for the AllToAll
                for h in range(2):
                    nc.vector.tensor_copy(ctx2_sb[:, h, q0:q0 + 512],
                                          cn[h][0:65, :])

            def emit_half_a2a(b, hf):
                base = b * S + hf * (S // 2)
                ctxd = dram.tile([N_CORES, CR, PH], BF16, tag="ctxd",
                                 name="ctxd", bufs=4)
                for j in range(N_CORES):
                    dst = ctxd[j]
                    nc.sync.dma_start(
                        out=bass.AP(tensor=dst.tensor, offset=dst.offset,
                                    ap=[[PH, 65], [65 * PH, 2], [1, PH]]),
                        in_=ctx2_sb[:, :, base + j * PH:base + (j + 1) * PH])
                recv = dram.tile([N_CORES, CR, PH], BF16, tag="recv",
                                 name="recv", bufs=4)
                nc.gpsimd.collective_compute(
                    "AllToAll",
                    mybir.AluOpType.bypass,
                    replica_groups=[list(range(N_CORES))],
                    ins=[ctxd.opt()],
                    outs=[recv.opt()],
                )
                return recv

            def emit_half_recv(b, hf, recv):
                # gather + normalize the received ctx; no PE work, so the PE
                # queue never parks on this chain
                r0 = recv[0]
                cg_sb = op.tile([128, NKC, PH], BF16, tag="cg_sb", name="cg_sb",
                                bufs=2)
                for h in range(2):
                    nc.sync.dma_start(
                        out=cg_sb[h * 64:(h + 1) * 64, :, :],
                        in_=bass.AP(tensor=r0.tensor,
                                    offset=r0.offset + h * 65 * PH,
                                    ap=[[PH, 64], [CR * PH, N_CORES],
                                        [1, PH]]))
                # 16 denominator rows stacked on partitions: p = 2*j + h
                den16 = op.tile([16, PH], BF16, tag="den16", name="den16",
                                bufs=2)
                nc.sync.dma_start(
                    out=den16[:],
                    in_=bass.AP(tensor=r0.tensor, offset=r0.offset + 64 * PH,
                                ap=[[CR * PH, N_CORES], [65 * PH, 2],
                                    [1, PH]]))
                r16 = op.tile([16, PH], F32, tag="r16", name="r16", bufs=2)
                nc.vector.reciprocal(r16[:], den16[:])
                rd = dram.tile([16, PH], F32, tag="rd", name="rd", bufs=4)
                nc.sync.dma_start(out=rd[:], in_=r16[:])
                rmap = op.tile([128, NKC, PH], F32, tag="rmap", name="rmap",
                               bufs=2)
                rd0 = rd[0]
                for h in range(2):
                    nc.sync.dma_start(
                        out=rmap[h * 64:(h + 1) * 64, :, :],
                        in_=bass.AP(tensor=rd0.tensor,
                                    offset=rd0.offset + h * PH,
                                    ap=[[0, 64], [2 * PH, N_CORES], [1, PH]]))
                nc.vector.tensor_mul(cg_sb[:], cg_sb[:], rmap[:])
                return b, hf, cg_sb

            def emit_half_wo(b, hf, cg_sb):
                o_sb = op.tile([PH, E], F32, tag="o_sb", name="o_sb")
                for et in range(2):
                    ps = ps_s.tile([128, 2, 512], F32, tag="s", name="c_ps")
                    for kc in range(NKC):
                        nc.tensor.matmul(
                            ps[0:PH, 0, :],
                            cg_sb[:, kc, :],
                            wo_sb[:, kc, et * 512:(et + 1) * 512],
                            start=(kc == 0), stop=(kc == NKC - 1),
                            skip_group_check=True)
                    nc.vector.tensor_add(
                        o_sb[:, et * 512:(et + 1) * 512], ps[0:PH, 0, :],
                        bo_bc[0:PH, et * 512:(et + 1) * 512])
                r0w = (b * 2 + hf) * PH
                nc.sync.dma_start(out=out[r0w:r0w + PH, :], in_=o_sb[:])

            # ---- interleaved schedule -------------------------------------
            # Per half-batch: A2A issued at its boundary; gather+normalize one
            # boundary later (collective long done); Wo matmuls one boundary
            # after that (inputs ready the moment the PE reaches them).
            a2aq = []  # (b, hf, recv): A2A issued, recv-chain not emitted
            woq = []   # (b, hf, cg_sb): normalized, Wo matmuls not emitted

            for b in range(B):
                for qt in range(4):
                    emit_proj(b * 4 + qt)
                    emit_attn(b, qt)
                    if qt in (1, 3):
                        if woq:
                            emit_half_wo(*woq.pop(0))
                        if a2aq:
                            woq.append(emit_half_recv(*a2aq.pop(0)))
                        a2aq.append((b, qt // 2,
                                     emit_half_a2a(b, qt // 2)))
            while woq or a2aq:
                if woq:
                    emit_half_wo(*woq.pop(0))
                if a2aq:
                    woq.append(emit_half_recv(*a2aq.pop(0)))

    nc.compile()
    return nc


_NC = None


def _get_program():
    global _NC
    if _NC is None:
        _NC = build_program()
    return _NC


def _bf(a):
    return np.ascontiguousarray(a).astype(ml_dtypes.bfloat16)


def kernel(x, Wq, bq, Wk, bk, Wv, bv, Wo, bo, _trace=False, _trace_kwargs=None):
    x = np.asarray(x, np.float32)
    Wq, Wk, Wv, Wo = (np.asarray(w, np.float32) for w in (Wq, Wk, Wv, Wo))
    bq, bk, bv, bo = (np.asarray(v, np.float32) for v in (bq, bk, bv, bo))

    xT = _bf(x.reshape(T, E).T)
    i = np.arange(128)
    tri = _bf((i[:, None] <= i[None, :]).astype(np.float32))
    ident = _bf(np.eye(128, dtype=np.float32))

    in_maps = []
    for c in range(N_CORES):
        sl = slice(c * DPC, (c + 1) * DPC)
        in_maps.append({
            "xT": xT,
            "wqT": _bf(Wq[sl, :].T),
            "wkT": _bf(Wk[sl, :].T),
            "wvT": _bf(Wv[sl, :].T),
            "woT": _bf(Wo.T),
            "bq": bq[sl].reshape(DPC, 1).copy(),
            "bk": bk[sl].reshape(DPC, 1).copy(),
            "bv": bv[sl].reshape(DPC, 1).copy(),
            "bo": bo,
            "tri": tri,
            "ident": ident,
        })

    nc = _get_program()
    res = run_bass_kernel_spmd(nc, in_maps, list(range(N_CORES)),
                               trace=_trace, **(_trace_kwargs or {}))
    # out[c] rows are [batch, half, 128]: row (b, hf, r) holds global
    # token b*2048 + hf*1024 + c*128 + r.
    stacked = np.stack([res.results[i]["out"].reshape(B, 2, 128, E)
                        for i in range(N_CORES)], axis=2)
    full = stacked.reshape(T, E)
    if _trace:
        return full.reshape(B, S, E), res
    return full.reshape(B, S, E)


# revision 34
# speedup vs baseline: 1.0753x; 1.0753x over previous
"""Multi-head attention (B=2, S=2048, H=16, D=64) on 8 Trainium2 NeuronCores.

Head-parallel tensor parallelism: core c owns heads {2c, 2c+1} (a 128-dim
slice of the model dim): column-parallel QKV projections and local causal
attention for its 2 heads, then an AllToAll of bf16 context vectors (one
half-batch at a time, pipelined behind attention) and a full-width Wo
projection for this core's own disjoint 128-token output slices.

Shaped by trace measurements on this part:

* x loads in 8 per-token-tile DMAs and the QKV projection of tile t is
  interleaved with attention on query group t-1, so the PE starts ~20 us
  earlier than with a monolithic x load, and ACT-bound attention stretches
  overlap projection matmuls.
* Attention-times-V keeps V plus a trailing ones column as the 65-column
  stationary operand and streams the exp tile (one matmul per key block,
  N<=512): context comes out already transposed ([dims, tokens]) and the
  softmax denominator lands on PSUM partition 64.
* Scores use tile_position row pairs: each head is a K=64 matmul on its own
  row-group half of the PE array, so the two heads' score matmuls run
  concurrently (no zero-padded K=128 operands).
* exp is one ACT instruction per key block covering both heads.
* Softmax normalization happens on the *receiving* core: the AllToAll
  payload is 130 rows per peer (65 per head: 64 unnormalized ctx dims plus
  the denominator row), so the 16 denominator rows stack on the partition
  axis at the receiver where one 16-lane DVE reciprocal + a DRAM-bounced
  stride-0 broadcast + one fused multiply normalize the gathered ctx.
  (DVE reciprocal is ~8 cycles/element/lane, so sender-side row-wise
  reciprocals were 3.3 us each; gpsimd partition_broadcast and the custom
  reciprocal_approx_fast DVE op both produce wrong results on hardware.)
* Collective triggers block the GpSimd queue until the collective
  completes, so nothing else is ever placed on GpSimd, and each half-batch
  Wo projection is emitted two sections after its AllToAll was issued.
* A tiny warm-up AllToAll is issued during the load phase so the first real
  collective doesn't pay the ~23 us first-call setup on the critical path.
"""

import sys

sys.path.insert(0, "/opt/trn_rl_repo")

import ml_dtypes
import numpy as np

import concourse.bass as bass
import concourse.tile as tile
from concourse import bacc, mybir
from concourse.bass_utils import run_bass_kernel_spmd

N_CORES = 8
B, S, H, D = 2, 2048, 16, 64
E = H * D            # 1024
T = B * S            # 4096 tokens
DPC = 128            # dims (2 heads) per core
NKC = E // 128       # 8 contraction chunks for the projections
NTT = T // 512       # 8 token tiles of 512
SB = S // 128        # 16 key blocks per batch
PH = S // 2 // N_CORES  # 128 tokens per core per half-batch
CR = 130             # a2a chunk rows: 2 x (64 ctx dims + den)

F32 = mybir.dt.float32
BF16 = mybir.dt.bfloat16
AFT = mybir.ActivationFunctionType


def build_program():
    nc = bacc.Bacc("TRN2", target_bir_lowering=False, debug=False,
                   num_devices=N_CORES)

    xT = nc.dram_tensor("xT", [E, T], BF16, kind="ExternalInput").ap()
    wqT = nc.dram_tensor("wqT", [E, DPC], BF16, kind="ExternalInput").ap()
    wkT = nc.dram_tensor("wkT", [E, DPC], BF16, kind="ExternalInput").ap()
    wvT = nc.dram_tensor("wvT", [E, DPC], BF16, kind="ExternalInput").ap()
    woT = nc.dram_tensor("woT", [E, E], BF16, kind="ExternalInput").ap()
    bq = nc.dram_tensor("bq", [DPC, 1], F32, kind="ExternalInput").ap()
    bk = nc.dram_tensor("bk", [DPC, 1], F32, kind="ExternalInput").ap()
    bv = nc.dram_tensor("bv", [DPC, 1], F32, kind="ExternalInput").ap()
    bo = nc.dram_tensor("bo", [E], F32, kind="ExternalInput").ap()
    # single 128x128 lower-triangular (k_local <= q_local) mask
    tri = nc.dram_tensor("tri", [128, 128], BF16, kind="ExternalInput").ap()
    ident = nc.dram_tensor("ident", [128, 128], BF16, kind="ExternalInput").ap()
    out = nc.dram_tensor("out", [T // N_CORES, E], F32, kind="ExternalOutput").ap()

    with tile.TileContext(nc) as tc:
        with (
            tc.tile_pool(name="consts", bufs=1) as consts,
            tc.tile_pool(name="state", bufs=1) as state,
            tc.tile_pool(name="ep", bufs=4) as ep,
            tc.tile_pool(name="op", bufs=4) as op,
            tc.tile_pool(name="ps_s", bufs=2, space="PSUM") as ps_s,
            tc.tile_pool(name="ps_c", bufs=3, space="PSUM") as ps_c,
            tc.tile_pool(name="ps_t", bufs=1, space="PSUM") as ps_t,
            tc.tile_pool(name="dram", bufs=1, space="DRAM") as dram,
        ):
            # ---- warm-up collective: absorbs the first-AllToAll setup cost
            # while the DMA engines are still loading x ----------------------
            wu_s = consts.tile([128, 16], BF16)
            nc.vector.memset(wu_s[:], 0.0)
            wu_in = dram.tile([N_CORES, 16, 16], BF16, tag="wu_in", name="wu_in")
            wu_out = dram.tile([N_CORES, 16, 16], BF16, tag="wu_out",
                               name="wu_out")
            nc.sync.dma_start(out=wu_in[:], in_=wu_s[:])
            nc.gpsimd.collective_compute(
                "AllToAll",
                mybir.AluOpType.bypass,
                replica_groups=[list(range(N_CORES))],
                ins=[wu_in.opt()],
                outs=[wu_out.opt()],
            )

            # ---- constants (one DMA per tensor) ---------------------------
            def chunked(dram_ap, cols):
                # DRAM [E, cols] viewed as [p, kc, cols]: row kc*128+p
                return bass.AP(tensor=dram_ap.tensor, offset=dram_ap.offset,
                               ap=[[cols, 128], [128 * cols, NKC], [1, cols]])

            wq_sb = consts.tile([128, NKC, DPC], BF16)
            wk_sb = consts.tile([128, NKC, DPC], BF16)
            wv_sb = consts.tile([128, NKC, DPC], BF16)
            nc.sync.dma_start(out=wq_sb[:], in_=chunked(wqT, DPC))
            nc.sync.dma_start(out=wk_sb[:], in_=chunked(wkT, DPC))
            nc.sync.dma_start(out=wv_sb[:], in_=chunked(wvT, DPC))
            bq_sb = consts.tile([128, 1], F32)
            bk_sb = consts.tile([128, 1], F32)
            bv_sb = consts.tile([128, 1], F32)
            nc.sync.dma_start(out=bq_sb[:], in_=bq[:])
            nc.sync.dma_start(out=bk_sb[:], in_=bk[:])
            nc.sync.dma_start(out=bv_sb[:], in_=bv[:])
            bo_bc = consts.tile([128, E], F32)
            nc.sync.dma_start(
                out=bo_bc[:],
                in_=bass.AP(tensor=bo.tensor, offset=bo.offset,
                            ap=[[0, 128], [1, E]]),
            )
            tri_sb = consts.tile([128, 128], BF16)
            nc.sync.dma_start(out=tri_sb[:], in_=tri[:])
            id_sb = consts.tile([128, 128], BF16)
            nc.sync.dma_start(out=id_sb[:], in_=ident[:])

            # ---- x, one tile per 512-token group --------------------------
            x_t = []
            for tt in range(NTT):
                xt = state.tile([128, NKC, 512], BF16, name=f"x{tt}")
                nc.sync.dma_start(
                    out=xt[:],
                    in_=bass.AP(tensor=xT.tensor, offset=xT.offset + tt * 512,
                                ap=[[T, 128], [128 * T, NKC], [1, 512]]))
                x_t.append(xt)
            wo_sb = consts.tile([128, NKC, E], BF16)
            nc.sync.dma_start(out=wo_sb[:], in_=chunked(woT, E))

            # ---- persistent activations -----------------------------------
            qT_sb = state.tile([128, T], BF16)   # [2-head dims, tokens]
            kT_sb = state.tile([128, T], BF16)
            vT_sb = state.tile([128, T], BF16)
            # per 128-token block: [64 v-dims, ones] per head -> the AV
            # matmul's 65-column stationary operand; the ones column makes
            # PSUM row 64 the softmax denominator.
            vN_sb = state.tile([128, T // 128, 130], BF16)
            # unnormalized ctx^T + den: rows 0-63 ctx dims, row 64 den
            ctx2_sb = state.tile([65, 2, T], BF16)

            nc.vector.memset(vN_sb[:, :, 64:65], 1.0)
            nc.vector.memset(vN_sb[:, :, 129:130], 1.0)

            # ---- stage builders -------------------------------------------
            def emit_proj(tt):
                ts = slice(tt * 512, (tt + 1) * 512)
                ps_qk = ps_s.tile([128, 2, 512], F32, tag="s", name="ps_qk")
                for kc in range(NKC):
                    nc.tensor.matmul(ps_qk[:, 0, :], wq_sb[:, kc, :],
                                     x_t[tt][:, kc, :],
                                     start=(kc == 0), stop=(kc == NKC - 1),
                                     skip_group_check=True)
                for kc in range(NKC):
                    nc.tensor.matmul(ps_qk[:, 1, :], wk_sb[:, kc, :],
                                     x_t[tt][:, kc, :],
                                     start=(kc == 0), stop=(kc == NKC - 1),
                                     skip_group_check=True)
                ps_v = ps_s.tile([128, 2, 512], F32, tag="s", name="ps_v")
                for kc in range(NKC):
                    nc.tensor.matmul(ps_v[:, 0, :], wv_sb[:, kc, :],
                                     x_t[tt][:, kc, :],
                                     start=(kc == 0), stop=(kc == NKC - 1),
                                     skip_group_check=True)
                nc.vector.tensor_scalar_add(qT_sb[:, ts], ps_qk[:, 0, :],
                                            bq_sb[:])
                nc.vector.tensor_scalar_add(kT_sb[:, ts], ps_qk[:, 1, :],
                                            bk_sb[:])
                nc.vector.tensor_scalar_add(vT_sb[:, ts], ps_v[:, 0, :],
                                            bv_sb[:])
                for tb in range(tt * 4, tt * 4 + 4):
                    tp_ps = ps_t.tile([128, 128], BF16, tag="tp", name="tp_ps")
                    nc.tensor.transpose(
                        tp_ps[:], vT_sb[:, tb * 128:(tb + 1) * 128], id_sb[:])
                    nc.vector.tensor_copy(vN_sb[:, tb, 0:64], tp_ps[:, 0:64])
                    nc.vector.tensor_copy(vN_sb[:, tb, 65:129],
                                          tp_ps[:, 64:128])

            def emit_attn(b, qt):
                t0 = b * S
                q0 = t0 + qt * 512
                nkb = 4 * qt + 4

                def emit_scores(kb):
                    c0 = max(kb - 4 * qt, 0) * 128
                    s = ps_s.tile([128, 2, 512], F32, tag="s", name="s_ps")
                    for h in range(2):
                        d0 = h * 64
                        nc.tensor.matmul(
                            s[:, h, c0:512],
                            kT_sb[d0:d0 + 64,
                                  t0 + kb * 128:t0 + (kb + 1) * 128],
                            qT_sb[d0:d0 + 64, q0 + c0:q0 + 512],
                            start=True, stop=True, skip_group_check=True)
                    return s

                s_tiles = {0: emit_scores(0)}
                cn = [ps_c.tile([128, 512], F32, tag="cn", name=f"cn{h}")
                      for h in range(2)]
                for kb in range(nkb):
                    m = kb - 4 * qt
                    c0 = max(m, 0) * 128
                    if kb + 1 < nkb:
                        s_tiles[kb + 1] = emit_scores(kb + 1)
                    s = s_tiles.pop(kb)
                    e = ep.tile([128, 2, 512], BF16, tag="e", name="e_sb")
                    nc.scalar.activation(e[:, :, c0:512], s[:, :, c0:512],
                                         AFT.Exp, scale=0.125)
                    if m >= 0:  # triangular block on the diagonal
                        for h in range(2):
                            nc.vector.tensor_mul(e[:, h, c0:c0 + 128],
                                                 e[:, h, c0:c0 + 128],
                                                 tri_sb[:])
                    for h in range(2):
                        nc.tensor.matmul(
                            cn[h][0:65, c0:512],
                            vN_sb[:, b * SB + kb, 65 * h:65 * h + 65],
                            e[:, h, c0:512],
                            start=(kb == 0), stop=(kb == nkb - 1),
                            skip_group_check=True)

                # stage unnormalized ctx + den rows for the AllToAll
                for h in range(2):
                    nc.vector.tensor_copy(ctx2_sb[:, h, q0:q0 + 512],
                                          cn[h][0:65, :])

            def emit_seg_a2a(base, w):
                ctxd = dram.tile([N_CORES, CR, w], BF16, tag=f"ctxd{w}",
                                 name="ctxd", bufs=4)
                for j in range(N_CORES):
                    dst = ctxd[j]
                    nc.sync.dma_start(
                        out=bass.AP(tensor=dst.tensor, offset=dst.offset,
                                    ap=[[w, 65], [65 * w, 2], [1, w]]),
                        in_=ctx2_sb[:, :, base + j * w:base + (j + 1) * w])
                recv = dram.tile([N_CORES, CR, w], BF16, tag=f"recv{w}",
                                 name="recv", bufs=4)
                nc.gpsimd.collective_compute(
                    "AllToAll",
                    mybir.AluOpType.bypass,
                    replica_groups=[list(range(N_CORES))],
                    ins=[ctxd.opt()],
                    outs=[recv.opt()],
                )
                return recv

            def emit_seg_recv(recv, w, out_r0):
                # gather + normalize the received ctx; no PE work, so the PE
                # queue never parks on this chain
                r0 = recv[0]
                cg_sb = op.tile([128, NKC, w], BF16, tag=f"cg_sb{w}",
                                name="cg_sb", bufs=2)
                for h in range(2):
                    nc.sync.dma_start(
                        out=cg_sb[h * 64:(h + 1) * 64, :, :],
                        in_=bass.AP(tensor=r0.tensor,
                                    offset=r0.offset + h * 65 * w,
                                    ap=[[w, 64], [CR * w, N_CORES],
                                        [1, w]]))
                # 16 denominator rows stacked on partitions: p = 2*j + h
                den16 = op.tile([16, w], BF16, tag=f"den16{w}", name="den16",
                                bufs=2)
                nc.sync.dma_start(
                    out=den16[:],
                    in_=bass.AP(tensor=r0.tensor, offset=r0.offset + 64 * w,
                                ap=[[CR * w, N_CORES], [65 * w, 2],
                                    [1, w]]))
                r16 = op.tile([16, w], F32, tag=f"r16{w}", name="r16", bufs=2)
                nc.vector.reciprocal(r16[:], den16[:])
                rd = dram.tile([16, w], F32, tag=f"rd{w}", name="rd", bufs=4)
                nc.sync.dma_start(out=rd[:], in_=r16[:])
                rmap = op.tile([128, NKC, w], F32, tag=f"rmap{w}", name="rmap",
                               bufs=2)
                rd0 = rd[0]
                for h in range(2):
                    nc.sync.dma_start(
                        out=rmap[h * 64:(h + 1) * 64, :, :],
                        in_=bass.AP(tensor=rd0.tensor,
                                    offset=rd0.offset + h * w,
                                    ap=[[0, 64], [2 * w, N_CORES], [1, w]]))
                nc.vector.tensor_mul(cg_sb[:], cg_sb[:], rmap[:])
                return cg_sb, w, out_r0

            def emit_seg_wo(cg_sb, w, out_r0):
                o_sb = op.tile([w, E], F32, tag=f"o_sb{w}", name="o_sb")
                for et in range(2):
                    ps = ps_s.tile([128, 2, 512], F32, tag="s", name="c_ps")
                    for kc in range(NKC):
                        nc.tensor.matmul(
                            ps[0:w, 0, :],
                            cg_sb[:, kc, :],
                            wo_sb[:, kc, et * 512:(et + 1) * 512],
                            start=(kc == 0), stop=(kc == NKC - 1),
                            skip_group_check=True)
                    nc.vector.tensor_add(
                        o_sb[:, et * 512:(et + 1) * 512], ps[0:w, 0, :],
                        bo_bc[0:w, et * 512:(et + 1) * 512])
                nc.sync.dma_start(out=out[out_r0:out_r0 + w, :], in_=o_sb[:])

            # ---- interleaved schedule -------------------------------------
            # Output segments: three 1024-token halves, then two 512-token
            # quarters so the final collective + projection tail is as short
            # as possible.  Per segment: A2A at its boundary, gather+normalize
            # one boundary later (collective long done), Wo matmuls one
            # boundary after that (inputs ready when the PE reaches them).
            segs = [(0, PH, 0), (1024, PH, 128), (2048, PH, 256),
                    (3072, 64, 384), (3584, 64, 448)]
            bounds = {(0, 1): 0, (0, 3): 1, (1, 1): 2, (1, 2): 3, (1, 3): 4}
            a2aq = []  # (recv, w, out_r0): A2A issued, recv-chain not emitted
            woq = []   # (cg_sb, w, out_r0): normalized, Wo matmuls not emitted

            for b in range(B):
                for qt in range(4):
                    emit_proj(b * 4 + qt)
                    emit_attn(b, qt)
                    if (b, qt) in bounds:
                        if woq:
                            emit_seg_wo(*woq.pop(0))
                        if a2aq:
                            woq.append(emit_seg_recv(*a2aq.pop(0)))
                        base, w, r0w = segs[bounds[(b, qt)]]
                        a2aq.append((emit_seg_a2a(base, w), w, r0w))
            while woq or a2aq:
                if woq:
                    emit_seg_wo(*woq.pop(0))
                if a2aq:
                    woq.append(emit_seg_recv(*a2aq.pop(0)))

    nc.compile()
    return nc


_NC = None


def _get_program():
    global _NC
    if _NC is None:
        _NC = build_program()
    return _NC


def _bf(a):
    return np.ascontiguousarray(a).astype(ml_dtypes.bfloat16)


def kernel(x, Wq, bq, Wk, bk, Wv, bv, Wo, bo, _trace=False, _trace_kwargs=None):
    x = np.asarray(x, np.float32)
    Wq, Wk, Wv, Wo = (np.asarray(w, np.float32) for w in (Wq, Wk, Wv, Wo))
    bq, bk, bv, bo = (np.asarray(v, np.float32) for v in (bq, bk, bv, bo))

    xT = _bf(x.reshape(T, E).T)
    i = np.arange(128)
    tri = _bf((i[:, None] <= i[None, :]).astype(np.float32))
    ident = _bf(np.eye(128, dtype=np.float32))

    in_maps = []
    for c in range(N_CORES):
        sl = slice(c * DPC, (c + 1) * DPC)
        in_maps.append({
            "xT": xT,
            "wqT": _bf(Wq[sl, :].T),
            "wkT": _bf(Wk[sl, :].T),
            "wvT": _bf(Wv[sl, :].T),
            "woT": _bf(Wo.T),
            "bq": bq[sl].reshape(DPC, 1).copy(),
            "bk": bk[sl].reshape(DPC, 1).copy(),
            "bv": bv[sl].reshape(DPC, 1).copy(),
            "bo": bo,
            "tri": tri,
            "ident": ident,
        })

    nc = _get_program()
    res = run_bass_kernel_spmd(nc, in_maps, list(range(N_CORES)),
                               trace=_trace, **(_trace_kwargs or {}))
    # out rows per core: segments of the token axis; core c owns the c-th
    # w-token slice of each segment (three 1024-token halves then two
    # 512-token quarters).
    full = np.empty((T, E), np.float32)
    for c in range(N_CORES):
        oc = res.results[c]["out"]
        for base, r0, w in ((0, 0, 128), (1024, 128, 128), (2048, 256, 128),
                            (3072, 384, 64), (3584, 448, 64)):
            full[base + c * w:base + (c + 1) * w] = oc[r0:r0 + w]
    if _trace:
        return full.reshape(B, S, E), res
    return full.reshape(B, S, E)


# revision 36
# speedup vs baseline: 1.1802x; 1.0975x over previous
"""Multi-head attention (B=2, S=2048, H=16, D=64) on 8 Trainium2 NeuronCores.

Head-parallel tensor parallelism: core c owns heads {2c, 2c+1} (a 128-dim
slice of the model dim): column-parallel QKV projections and local causal
attention for its 2 heads, then an AllToAll of bf16 context vectors (one
half-batch at a time, pipelined behind attention) and a full-width Wo
projection for this core's own disjoint 128-token output slices.

Shaped by trace measurements on this part:

* x loads in 8 per-token-tile DMAs and the QKV projection of tile t is
  interleaved with attention on query group t-1, so the PE starts ~20 us
  earlier than with a monolithic x load, and ACT-bound attention stretches
  overlap projection matmuls.
* Attention-times-V keeps V plus a trailing ones column as the 65-column
  stationary operand and streams the exp tile (one matmul per key block,
  N<=512): context comes out already transposed ([dims, tokens]) and the
  softmax denominator lands on PSUM partition 64.
* Scores use tile_position row pairs: each head is a K=64 matmul on its own
  row-group half of the PE array, so the two heads' score matmuls run
  concurrently (no zero-padded K=128 operands).
* exp is one ACT instruction per key block covering both heads.
* Softmax normalization happens on the *receiving* core: the AllToAll
  payload is 130 rows per peer (65 per head: 64 unnormalized ctx dims plus
  the denominator row), so the 16 denominator rows stack on the partition
  axis at the receiver where one 16-lane DVE reciprocal + a DRAM-bounced
  stride-0 broadcast + one fused multiply normalize the gathered ctx.
  (DVE reciprocal is ~8 cycles/element/lane, so sender-side row-wise
  reciprocals were 3.3 us each; gpsimd partition_broadcast and the custom
  reciprocal_approx_fast DVE op both produce wrong results on hardware.)
* Collective triggers block the GpSimd queue until the collective
  completes, so nothing else is ever placed on GpSimd, and each half-batch
  Wo projection is emitted two sections after its AllToAll was issued.
* A tiny warm-up AllToAll is issued during the load phase so the first real
  collective doesn't pay the ~23 us first-call setup on the critical path.
"""

import sys

sys.path.insert(0, "/opt/trn_rl_repo")

import ml_dtypes
import numpy as np

import concourse.bass as bass
import concourse.tile as tile
from concourse import bacc, mybir
from concourse.bass_utils import run_bass_kernel_spmd

N_CORES = 8
B, S, H, D = 2, 2048, 16, 64
E = H * D            # 1024
T = B * S            # 4096 tokens
DPC = 128            # dims (2 heads) per core
NKC = E // 128       # 8 contraction chunks for the projections
NTT = T // 512       # 8 token tiles of 512
SB = S // 128        # 16 key blocks per batch
PH = S // 2 // N_CORES  # 128 tokens per core per half-batch
CR = 130             # a2a chunk rows: 2 x (64 ctx dims + den)

F32 = mybir.dt.float32
BF16 = mybir.dt.bfloat16
AFT = mybir.ActivationFunctionType


def build_program():
    nc = bacc.Bacc("TRN2", target_bir_lowering=False, debug=False,
                   num_devices=N_CORES)

    xT = nc.dram_tensor("xT", [E, T], BF16, kind="ExternalInput").ap()
    wqT = nc.dram_tensor("wqT", [E, DPC], BF16, kind="ExternalInput").ap()
    wkT = nc.dram_tensor("wkT", [E, DPC], BF16, kind="ExternalInput").ap()
    wvT = nc.dram_tensor("wvT", [E, DPC], BF16, kind="ExternalInput").ap()
    woT = nc.dram_tensor("woT", [E, E], BF16, kind="ExternalInput").ap()
    bq = nc.dram_tensor("bq", [DPC, 1], F32, kind="ExternalInput").ap()
    bk = nc.dram_tensor("bk", [DPC, 1], F32, kind="ExternalInput").ap()
    bv = nc.dram_tensor("bv", [DPC, 1], F32, kind="ExternalInput").ap()
    bo = nc.dram_tensor("bo", [E], F32, kind="ExternalInput").ap()
    # single 128x128 lower-triangular (k_local <= q_local) mask
    tri = nc.dram_tensor("tri", [128, 128], BF16, kind="ExternalInput").ap()
    ident = nc.dram_tensor("ident", [128, 128], BF16, kind="ExternalInput").ap()
    out = nc.dram_tensor("out", [T // N_CORES, E], F32, kind="ExternalOutput").ap()

    with tile.TileContext(nc) as tc:
        with (
            tc.tile_pool(name="consts", bufs=1) as consts,
            tc.tile_pool(name="state", bufs=1) as state,
            tc.tile_pool(name="ep", bufs=6) as ep,
            tc.tile_pool(name="op", bufs=4) as op,
            tc.tile_pool(name="ps_s", bufs=2, space="PSUM") as ps_s,
            tc.tile_pool(name="ps_c", bufs=3, space="PSUM") as ps_c,
            tc.tile_pool(name="ps_t", bufs=1, space="PSUM") as ps_t,
            tc.tile_pool(name="dram", bufs=1, space="DRAM") as dram,
        ):
            # ---- warm-up collective: absorbs the first-AllToAll setup cost
            # while the DMA engines are still loading x ----------------------
            wu_s = consts.tile([128, 16], BF16)
            nc.vector.memset(wu_s[:], 0.0)
            wu_in = dram.tile([N_CORES, 16, 16], BF16, tag="wu_in", name="wu_in")
            wu_out = dram.tile([N_CORES, 16, 16], BF16, tag="wu_out",
                               name="wu_out")
            nc.sync.dma_start(out=wu_in[:], in_=wu_s[:])
            nc.gpsimd.collective_compute(
                "AllToAll",
                mybir.AluOpType.bypass,
                replica_groups=[list(range(N_CORES))],
                ins=[wu_in.opt()],
                outs=[wu_out.opt()],
            )

            # ---- constants (one DMA per tensor) ---------------------------
            def chunked(dram_ap, cols):
                # DRAM [E, cols] viewed as [p, kc, cols]: row kc*128+p
                return bass.AP(tensor=dram_ap.tensor, offset=dram_ap.offset,
                               ap=[[cols, 128], [128 * cols, NKC], [1, cols]])

            wq_sb = consts.tile([128, NKC, DPC], BF16)
            wk_sb = consts.tile([128, NKC, DPC], BF16)
            wv_sb = consts.tile([128, NKC, DPC], BF16)
            nc.sync.dma_start(out=wq_sb[:], in_=chunked(wqT, DPC))
            nc.sync.dma_start(out=wk_sb[:], in_=chunked(wkT, DPC))
            nc.sync.dma_start(out=wv_sb[:], in_=chunked(wvT, DPC))
            bq_sb = consts.tile([128, 1], F32)
            bk_sb = consts.tile([128, 1], F32)
            bv_sb = consts.tile([128, 1], F32)
            nc.sync.dma_start(out=bq_sb[:], in_=bq[:])
            nc.sync.dma_start(out=bk_sb[:], in_=bk[:])
            nc.sync.dma_start(out=bv_sb[:], in_=bv[:])
            bo_bc = consts.tile([128, E], F32)
            nc.sync.dma_start(
                out=bo_bc[:],
                in_=bass.AP(tensor=bo.tensor, offset=bo.offset,
                            ap=[[0, 128], [1, E]]),
            )
            tri_sb = consts.tile([128, 128], BF16)
            nc.sync.dma_start(out=tri_sb[:], in_=tri[:])
            id_sb = consts.tile([128, 128], BF16)
            nc.sync.dma_start(out=id_sb[:], in_=ident[:])

            # ---- x, one tile per 512-token group.  DMA issue order
            # interleaves batch-0 and batch-1 tiles (0,4,1,5,...) so batch
            # 1's tiles land before their projections need them instead of
            # draining last behind all of batch 0's bulk. ------------------
            x_t = [None] * NTT
            for tt in (0, 4, 1, 5, 2, 6, 3, 7):
                xt = state.tile([128, NKC, 512], BF16, name=f"x{tt}")
                nc.sync.dma_start(
                    out=xt[:],
                    in_=bass.AP(tensor=xT.tensor, offset=xT.offset + tt * 512,
                                ap=[[T, 128], [128 * T, NKC], [1, 512]]))
                x_t[tt] = xt
            wo_sb = consts.tile([128, NKC, E], BF16)
            nc.sync.dma_start(out=wo_sb[:], in_=chunked(woT, E))

            # ---- persistent activations -----------------------------------
            qT_sb = state.tile([128, T], BF16)   # [2-head dims, tokens]
            kT_sb = state.tile([128, T], BF16)
            vT_sb = state.tile([128, T], BF16)
            # per 128-token block: [64 v-dims, ones] per head -> the AV
            # matmul's 65-column stationary operand; the ones column makes
            # PSUM row 64 the softmax denominator.
            vN_sb = state.tile([128, T // 128, 130], BF16)
            # unnormalized ctx^T + den: rows 0-63 ctx dims, row 64 den
            ctx2_sb = state.tile([65, 2, T], BF16)

            nc.vector.memset(vN_sb[:, :, 64:65], 1.0)
            nc.vector.memset(vN_sb[:, :, 129:130], 1.0)

            # ---- stage builders -------------------------------------------
            def emit_proj(tt):
                ts = slice(tt * 512, (tt + 1) * 512)
                ps_qk = ps_s.tile([128, 2, 512], F32, tag="s", name="ps_qk")
                for kc in range(NKC):
                    nc.tensor.matmul(ps_qk[:, 0, :], wq_sb[:, kc, :],
                                     x_t[tt][:, kc, :],
                                     start=(kc == 0), stop=(kc == NKC - 1),
                                     skip_group_check=True)
                for kc in range(NKC):
                    nc.tensor.matmul(ps_qk[:, 1, :], wk_sb[:, kc, :],
                                     x_t[tt][:, kc, :],
                                     start=(kc == 0), stop=(kc == NKC - 1),
                                     skip_group_check=True)
                ps_v = ps_s.tile([128, 2, 512], F32, tag="s", name="ps_v")
                for kc in range(NKC):
                    nc.tensor.matmul(ps_v[:, 0, :], wv_sb[:, kc, :],
                                     x_t[tt][:, kc, :],
                                     start=(kc == 0), stop=(kc == NKC - 1),
                                     skip_group_check=True)
                nc.vector.tensor_scalar_add(qT_sb[:, ts], ps_qk[:, 0, :],
                                            bq_sb[:])
                nc.vector.tensor_scalar_add(kT_sb[:, ts], ps_qk[:, 1, :],
                                            bk_sb[:])
                nc.vector.tensor_scalar_add(vT_sb[:, ts], ps_v[:, 0, :],
                                            bv_sb[:])
                for tb in range(tt * 4, tt * 4 + 4):
                    tp_ps = ps_t.tile([128, 128], BF16, tag="tp", name="tp_ps")
                    nc.tensor.transpose(
                        tp_ps[:], vT_sb[:, tb * 128:(tb + 1) * 128], id_sb[:])
                    nc.vector.tensor_copy(vN_sb[:, tb, 0:64], tp_ps[:, 0:64])
                    nc.vector.tensor_copy(vN_sb[:, tb, 65:129],
                                          tp_ps[:, 64:128])

            def emit_attn(b, qt):
                t0 = b * S
                q0 = t0 + qt * 512
                nkb = 4 * qt + 4

                def emit_scores(kb):
                    c0 = max(kb - 4 * qt, 0) * 128
                    s = ps_s.tile([128, 2, 512], F32, tag="s", name="s_ps")
                    for h in range(2):
                        d0 = h * 64
                        nc.tensor.matmul(
                            s[:, h, c0:512],
                            kT_sb[d0:d0 + 64,
                                  t0 + kb * 128:t0 + (kb + 1) * 128],
                            qT_sb[d0:d0 + 64, q0 + c0:q0 + 512],
                            start=True, stop=True, skip_group_check=True)
                    return s

                s_tiles = {0: emit_scores(0)}
                cn = [ps_c.tile([128, 512], F32, tag="cn", name=f"cn{h}")
                      for h in range(2)]
                for kb in range(nkb):
                    m = kb - 4 * qt
                    c0 = max(m, 0) * 128
                    if kb + 1 < nkb:
                        s_tiles[kb + 1] = emit_scores(kb + 1)
                    s = s_tiles.pop(kb)
                    e = ep.tile([128, 2, 512], BF16, tag="e", name="e_sb")
                    nc.scalar.activation(e[:, :, c0:512], s[:, :, c0:512],
                                         AFT.Exp, scale=0.125)
                    if m >= 0:  # triangular block on the diagonal
                        for h in range(2):
                            nc.vector.tensor_mul(e[:, h, c0:c0 + 128],
                                                 e[:, h, c0:c0 + 128],
                                                 tri_sb[:])
                    for h in range(2):
                        nc.tensor.matmul(
                            cn[h][0:65, c0:512],
                            vN_sb[:, b * SB + kb, 65 * h:65 * h + 65],
                            e[:, h, c0:512],
                            start=(kb == 0), stop=(kb == nkb - 1),
                            skip_group_check=True)

                # stage unnormalized ctx + den rows for the AllToAll
                for h in range(2):
                    nc.vector.tensor_copy(ctx2_sb[:, h, q0:q0 + 512],
                                          cn[h][0:65, :])

            def emit_half_a2a(b, hf):
                base = b * S + hf * (S // 2)
                ctxd = dram.tile([N_CORES, CR, PH], BF16, tag="ctxd",
                                 name="ctxd", bufs=4)
                for j in range(N_CORES):
                    dst = ctxd[j]
                    nc.sync.dma_start(
                        out=bass.AP(tensor=dst.tensor, offset=dst.offset,
                                    ap=[[PH, 65], [65 * PH, 2], [1, PH]]),
                        in_=ctx2_sb[:, :, base + j * PH:base + (j + 1) * PH])
                recv = dram.tile([N_CORES, CR, PH], BF16, tag="recv",
                                 name="recv", bufs=4)
                nc.gpsimd.collective_compute(
                    "AllToAll",
                    mybir.AluOpType.bypass,
                    replica_groups=[list(range(N_CORES))],
                    ins=[ctxd.opt()],
                    outs=[recv.opt()],
                )
                return recv

            def emit_half_recv(b, hf, recv):
                # gather + normalize the received ctx; no PE work, so the PE
                # queue never parks on this chain
                r0 = recv[0]
                cg_sb = op.tile([128, NKC, PH], BF16, tag="cg_sb", name="cg_sb",
                                bufs=2)
                for h in range(2):
                    nc.sync.dma_start(
                        out=cg_sb[h * 64:(h + 1) * 64, :, :],
                        in_=bass.AP(tensor=r0.tensor,
                                    offset=r0.offset + h * 65 * PH,
                                    ap=[[PH, 64], [CR * PH, N_CORES],
                                        [1, PH]]))
                # 16 denominator rows stacked on partitions: p = 2*j + h
                den16 = op.tile([16, PH], BF16, tag="den16", name="den16",
                                bufs=2)
                nc.sync.dma_start(
                    out=den16[:],
                    in_=bass.AP(tensor=r0.tensor, offset=r0.offset + 64 * PH,
                                ap=[[CR * PH, N_CORES], [65 * PH, 2],
                                    [1, PH]]))
                r16 = op.tile([16, PH], F32, tag="r16", name="r16", bufs=2)
                nc.vector.reciprocal(r16[:], den16[:])
                rd = dram.tile([16, PH], F32, tag="rd", name="rd", bufs=4)
                nc.sync.dma_start(out=rd[:], in_=r16[:])
                rmap = op.tile([128, NKC, PH], F32, tag="rmap", name="rmap",
                               bufs=2)
                rd0 = rd[0]
                for h in range(2):
                    nc.sync.dma_start(
                        out=rmap[h * 64:(h + 1) * 64, :, :],
                        in_=bass.AP(tensor=rd0.tensor,
                                    offset=rd0.offset + h * PH,
                                    ap=[[0, 64], [2 * PH, N_CORES], [1, PH]]))
                nc.vector.tensor_mul(cg_sb[:], cg_sb[:], rmap[:])
                return b, hf, cg_sb

            def emit_half_wo(b, hf, cg_sb):
                o_sb = op.tile([PH, E], F32, tag="o_sb", name="o_sb")
                for et in range(2):
                    ps = ps_s.tile([128, 2, 512], F32, tag="s", name="c_ps")
                    for kc in range(NKC):
                        nc.tensor.matmul(
                            ps[0:PH, 0, :],
                            cg_sb[:, kc, :],
                            wo_sb[:, kc, et * 512:(et + 1) * 512],
                            start=(kc == 0), stop=(kc == NKC - 1),
                            skip_group_check=True)
                    nc.vector.tensor_add(
                        o_sb[:, et * 512:(et + 1) * 512], ps[0:PH, 0, :],
                        bo_bc[0:PH, et * 512:(et + 1) * 512])
                r0w = (b * 2 + hf) * PH
                nc.sync.dma_start(out=out[r0w:r0w + PH, :], in_=o_sb[:])

            # ---- interleaved schedule -------------------------------------
            # Per half-batch: A2A issued at its boundary; gather+normalize one
            # boundary later (collective long done); Wo matmuls one boundary
            # after that (inputs ready the moment the PE reaches them).
            a2aq = []  # (b, hf, recv): A2A issued, recv-chain not emitted
            woq = []   # (b, hf, cg_sb): normalized, Wo matmuls not emitted

            for b in range(B):
                for qt in range(4):
                    emit_proj(b * 4 + qt)
                    emit_attn(b, qt)
                    if qt in (1, 3):
                        if woq:
                            emit_half_wo(*woq.pop(0))
                        if a2aq:
                            woq.append(emit_half_recv(*a2aq.pop(0)))
                        a2aq.append((b, qt // 2,
                                     emit_half_a2a(b, qt // 2)))
            while woq or a2aq:
                if woq:
                    emit_half_wo(*woq.pop(0))
                if a2aq:
                    woq.append(emit_half_recv(*a2aq.pop(0)))

    nc.compile()
    return nc


_NC = None


def _get_program():
    global _NC
    if _NC is None:
        _NC = build_program()
    return _NC


def _bf(a):
    return np.ascontiguousarray(a).astype(ml_dtypes.bfloat16)


def kernel(x, Wq, bq, Wk, bk, Wv, bv, Wo, bo, _trace=False, _trace_kwargs=None):
    x = np.asarray(x, np.float32)
    Wq, Wk, Wv, Wo = (np.asarray(w, np.float32) for w in (Wq, Wk, Wv, Wo))
    bq, bk, bv, bo = (np.asarray(v, np.float32) for v in (bq, bk, bv, bo))

    xT = _bf(x.reshape(T, E).T)
    i = np.arange(128)
    tri = _bf((i[:, None] <= i[None, :]).astype(np.float32))
    ident = _bf(np.eye(128, dtype=np.float32))

    in_maps = []
    for c in range(N_CORES):
        sl = slice(c * DPC, (c + 1) * DPC)
        in_maps.append({
            "xT": xT,
            "wqT": _bf(Wq[sl, :].T),
            "wkT": _bf(Wk[sl, :].T),
            "wvT": _bf(Wv[sl, :].T),
            "woT": _bf(Wo.T),
            "bq": bq[sl].reshape(DPC, 1).copy(),
            "bk": bk[sl].reshape(DPC, 1).copy(),
            "bv": bv[sl].reshape(DPC, 1).copy(),
            "bo": bo,
            "tri": tri,
            "ident": ident,
        })

    nc = _get_program()
    res = run_bass_kernel_spmd(nc, in_maps, list(range(N_CORES)),
                               trace=_trace, **(_trace_kwargs or {}))
    # out[c] rows are [batch, half, 128]: row (b, hf, r) holds global
    # token b*2048 + hf*1024 + c*128 + r.
    stacked = np.stack([res.results[i]["out"].reshape(B, 2, 128, E)
                        for i in range(N_CORES)], axis=2)
    full = stacked.reshape(T, E)
    if _trace:
        return full.reshape(B, S, E), res
    return full.reshape(B, S, E)
